# revision 9
# baseline (speedup 1.0000x reference)
"""Trainium2 Bass kernel for nn_CausalSelfAttention (B=2, T=4096, D=512, H=8, hd=64).

Sharding: batch x head-pair over 8 cores (core i: batch i//4, heads 2*(i%4), 2*(i%4)+1).
Each core computes QKV projection + RoPE + full-T causal attention for its 2 heads and
a partial output projection (row-parallel c_proj); host sums the 4 partials per batch.

Design (v5). Two engine streams dominate and must both stay saturated:
  - ACT (scalar) runs softmax exp over all T^2/2 scores: ~123us of streaming
    at 1 elem/cycle/lane plus per-instruction overhead; exp groups of 2
    128-k-chunks from fp32 PSUM [128,1024].
  - PE runs S = K^T Q chunks, AV accumulation (ones-augmented V gives the
    softmax denominator for free), QKV projections, RoPE rotate-half as one
    [128x128] permutation matmul (sign folded into host-prepped ss), V^T
    computed directly in transposed form, and the y projection.
The S->exp->S chain is decoupled with THREE s-psum buffers (runway of 3
exp groups) so ACT never starves; AV lags LAG groups behind exp. Projections
for block Jb+1 and the y projection of Jb-1 are emitted as filler pieces
between attention groups of block Jb so the in-order PE queue stays dense
and the HAM clock gate stays released. Normalization: 1/l via
reciprocal_approx_fast on an SBUF copy of the PSUM ones-row, gpsimd
partition_broadcast, one TT-mul writing fp16. y is DMA'd out as fp16; the
host only transposes and accumulates the 4 head-pair partials per batch.
PSUM: s 3x2 + o 1 + aux 1 = 8 banks (s tiles [128,1024]f32 = 2 banks each).
"""

import sys

sys.path.insert(0, "/opt/trn_rl_repo")

from collections import deque
from contextlib import ExitStack

import ml_dtypes
import numpy as np

import concourse.bass as bass
import concourse.tile as tile
from concourse import bacc, mybir
from concourse.bass import ts
from concourse.bass_utils import run_bass_kernel_spmd

F32 = mybir.dt.float32
F16 = mybir.dt.float16

B, C, H, HD = 2, 512, 8, 64
N_CORES = 8


def build_kernel(T=4096, n_cores=N_CORES):
    nc = bacc.Bacc(
        "TRN2",
        target_bir_lowering=False,
        debug=False,
        num_devices=n_cores,
    )
    NJ = T // 512
    NK = T // 128
    QB = 512
    NB = T // QB
    LAG = 2
    GS = 2

    xT_d = nc.dram_tensor("xT", [C, T], F16, kind="ExternalInput").ap()
    cc_d = nc.dram_tensor("ccT", [128, T], F16, kind="ExternalInput").ap()
    ss_d = nc.dram_tensor("ssT", [128, T], F16, kind="ExternalInput").ap()
    w_d = {}
    for name in ("wqT", "wkT", "wvT"):
        w_d[name] = nc.dram_tensor(name, [C, 128], F16, kind="ExternalInput").ap()
    wp_d = nc.dram_tensor("wpT", [128, C], F16, kind="ExternalInput").ap()
    msk_d = nc.dram_tensor("masks", [128, 4, QB], F16, kind="ExternalInput").ap()
    perm_d = nc.dram_tensor("perm", [128, 128], F16, kind="ExternalInput").ap()
    y_d = nc.dram_tensor("yT", [C, T], F16, kind="ExternalOutput").ap()
    warm_d = nc.dram_tensor("warm", [1, 4], F32, kind="ExternalOutput").ap()

    SCALE = float(1.0 / np.sqrt(HD))

    with tile.TileContext(nc) as tc, ExitStack() as ctx:
        consts = ctx.enter_context(tc.tile_pool(name="consts", bufs=1))
        big = ctx.enter_context(tc.tile_pool(name="big", bufs=1))
        xpool = ctx.enter_context(tc.tile_pool(name="xpool", bufs=3))
        qpool = ctx.enter_context(tc.tile_pool(name="qpool", bufs=4))
        rpool = ctx.enter_context(tc.tile_pool(name="rpool", bufs=6))
        epool = ctx.enter_context(tc.tile_pool(name="epool", bufs=5))
        opool = ctx.enter_context(tc.tile_pool(name="opool", bufs=3))
        spool = ctx.enter_context(tc.tile_pool(name="small", bufs=4))
        ypool = ctx.enter_context(tc.tile_pool(name="ypool", bufs=2))

        ps_aux = ctx.enter_context(tc.tile_pool(name="ps_aux", bufs=1, space="PSUM"))
        ps_s = ctx.enter_context(tc.tile_pool(name="ps_s", bufs=3, space="PSUM"))
        ps_o = ctx.enter_context(tc.tile_pool(name="ps_o", bufs=1, space="PSUM"))

        # ---- PE warmup burst first: matmuls on a small memset tile release
        # the HAM clock gate while the first DMAs land. Emitted before any
        # other DVE work so the wz memset is at the head of the DVE queue.
        wz = spool.tile([128, 512], F16, tag="wz")
        nc.vector.memset(wz[:], 0.25)
        wu_ps = ps_aux.tile([128, 512], F32, tag="p")
        for _ in range(6):
            nc.tensor.matmul(wu_ps[:], wz[:, 0:128], wz[:], start=True, stop=True)
        # preload the exp table set while ACT is otherwise idle
        wexp = spool.tile([1, 4], F16, tag="wexp")
        nc.scalar.activation(wexp[:], wu_ps[0:1, 0:4],
                             mybir.ActivationFunctionType.Exp, scale=0.001)
        wsink = spool.tile([1, 4], F32, tag="wsink")
        nc.vector.tensor_copy(wsink[:], wu_ps[0:1, 0:4])
        nc.sync.dma_start(warm_d[:], wsink[:])

        # weights + rope tables on the gpsimd DMA queue (parallel with the
        # x loads on the sync queue)
        w_sb = {}
        for name in ("wqT", "wkT", "wvT"):
            w = consts.tile([128, 4, 128], F16, tag=name, name=f"w_{name}")
            nc.gpsimd.dma_start(w[:], w_d[name].rearrange("(c p) m -> p c m", c=4))
            w_sb[name] = w
        cc = consts.tile([128, T], F16, name="cc")
        ss = consts.tile([128, T], F16, name="ss")
        nc.gpsimd.dma_start(cc[:], cc_d[:])
        nc.gpsimd.dma_start(ss[:], ss_d[:])
        perm = consts.tile([128, 128], F16)
        nc.gpsimd.dma_start(perm[:], perm_d[:])

        krT = big.tile([128, T], F16)
        v_aug = big.tile([128, 2, NK, 65], F16)
        nc.vector.memset(v_aug[:], 1.0)

        masks = consts.tile([128, 4, QB], F16, name="masks")
        nc.gpsimd.dma_start(masks[:], msk_d[:])
        w_p = consts.tile([128, C], F16, name="wp")
        nc.gpsimd.dma_start(w_p[:], wp_d[:])

        qr_tiles = {}
        o_tiles = {}

        def proj_pieces(j):
            """Emit-able pieces of the j-th projection block (label j). Each
            piece is a short PE burst; DVE/DMA consumers run while later
            pieces and surrounding attention groups keep the PE busy."""
            jc = ts(j, 512)
            st = {}

            def p_x():
                xc = xpool.tile([128, 4, 512], F16, tag="xc")
                nc.sync.dma_start(xc[:], xT_d.rearrange("(c p) t -> p c t", c=4)[:, :, jc])
                st["xc"] = xc

            def mk_qk(name, out_tag):
                def piece():
                    ps = ps_aux.tile([128, 512], F32, tag="p", name=f"ps_{name}_{j}")
                    for c in range(4):
                        nc.tensor.matmul(
                            ps[:], w_sb[name][:, c, :], st["xc"][:, c, :],
                            start=(c == 0), stop=(c == 3),
                        )
                    a_sb = qpool.tile([128, 512], F16, tag="a")
                    nc.vector.tensor_copy(a_sb[:], ps[:])
                    st[out_tag] = a_sb
                return piece

            def mk_rope(a_tag, out_name):
                def piece():
                    # qb = perm.T @ qa (the rotate-half partition swap on PE);
                    # m2 reads it straight from PSUM
                    b_ps = ps_aux.tile([128, 512], F32, tag="p", name=f"ps_b_{out_name}_{j}")
                    nc.tensor.matmul(b_ps[:], perm[:], st[a_tag][:], start=True, stop=True)
                    m1 = rpool.tile([128, 512], F16, tag="m1")
                    m2 = rpool.tile([128, 512], F16, tag="m2")
                    nc.vector.tensor_mul(m1[:], st[a_tag][:], cc[:, jc])
                    nc.vector.tensor_mul(m2[:], b_ps[:], ss[:, jc])
                    if out_name == "q":
                        qr = qpool.tile([128, 512], F16, tag="qr", name=f"qr_{j}")
                        nc.vector.tensor_add(qr[:], m1[:], m2[:])
                        qr_tiles[j] = qr
                    else:
                        nc.vector.tensor_add(krT[:, jc], m1[:], m2[:])
                return piece

            def p_vt():
                vt_ps = ps_aux.tile([128, 512], F32, tag="p", name=f"ps_vt_{j}")
                for kc in range(4):
                    for c in range(4):
                        nc.tensor.matmul(
                            vt_ps[:, ts(kc, 128)],
                            st["xc"][:, c, ts(kc, 128)], w_sb["wvT"][:, c, :],
                            start=(c == 0), stop=(c == 3),
                        )
                vw = vt_ps[:].rearrange("p (kc h d) -> p kc h d", kc=4, h=2)
                for hh in range(2):
                    nc.vector.tensor_copy(
                        v_aug[:, hh, 4 * j : 4 * j + 4, 0:64], vw[:, :, hh, :]
                    )

            return [
                p_x,
                mk_qk("wqT", "qa"),
                mk_rope("qa", "q"),
                mk_qk("wkT", "ka"),
                mk_rope("ka", "k"),
                p_vt,
            ]

        def y_pieces(Jb):
            jc = ts(Jb, QB)

            def mk(c):
                def piece():
                    oT = o_tiles[Jb]
                    if c == 3:
                        o_tiles.pop(Jb)
                    y_ps = ps_aux.tile([128, QB], F32, tag="p", name=f"ps_y_{Jb}_{c}")
                    nc.tensor.matmul(
                        y_ps[:], w_p[:, ts(c, 128)], oT[:], start=True, stop=True
                    )
                    y_sb = ypool.tile([128, QB], F16, tag="ysb")
                    nc.vector.tensor_copy(y_sb[:], y_ps[:])
                    nc.sync.dma_start(y_d[ts(c, 128), jc], y_sb[:])
                return piece

            return [mk(0), mk(1), mk(2), mk(3)]

        fillers = deque()  # entries: (label, piece)

        def pump(drain_j=None):
            if drain_j is None:
                if fillers:
                    fillers.popleft()[1]()
            else:
                while any(lbl == drain_j for lbl, _ in fillers):
                    fillers.popleft()[1]()

        for piece in proj_pieces(0):
            piece()
        fillers.extend((1, p) for p in proj_pieces(1))

        for Jb in range(NB):
            jc = ts(Jb, QB)
            if Jb + 1 < NJ:
                fillers.extend((Jb + 1, p) for p in proj_pieces(Jb + 1))
            pump(drain_j=Jb)  # everything this block depends on is now emitted
            for h in range(2):
                r = 64 * h
                nchunks = 4 * (Jb + 1)
                ngroups = nchunks // GS
                o_ps = ps_o.tile([65, QB], F32, tag="o", name=f"ps_o_{Jb}_{h}")
                e_tiles = {}
                qr = qr_tiles[Jb]

                def emit_av(g):
                    e_sb = e_tiles.pop(g)
                    for ci in range(GS):
                        cci = GS * g + ci
                        nc.tensor.matmul(
                            o_ps[:],
                            v_aug[:, h, cci, :],
                            e_sb[:, ts(ci, QB)],
                            start=(cci == 0),
                            stop=(cci == nchunks - 1),
                        )

                for g in range(ngroups + LAG):
                    if g < ngroups:
                        s_ps = ps_s.tile([128, GS * QB], F32, tag="s", name=f"ps_s_{Jb}_{h}_{g}")
                        for ci in range(GS):
                            cci = GS * g + ci
                            nc.tensor.matmul(
                                s_ps[:, ts(ci, QB)],
                                krT[r : r + 64, ts(cci, 128)],
                                qr[r : r + 64, :],
                                start=True,
                                stop=True,
                            )
                        e_sb = epool.tile([128, GS * QB], F16, tag="e")
                        nc.scalar.activation(
                            e_sb[:], s_ps[:], mybir.ActivationFunctionType.Exp,
                            scale=SCALE,
                        )
                        for ci in range(GS):
                            m = GS * g + ci - 4 * Jb
                            if m >= 0:
                                nc.vector.tensor_mul(
                                    e_sb[:, ts(ci, QB)], e_sb[:, ts(ci, QB)],
                                    masks[:, m, :],
                                )
                        e_tiles[g] = e_sb
                    if g >= LAG:
                        emit_av(g - LAG)
                    pump()

                # normalize: oT[h] = o * (1/l)
                if h == 0:
                    oT = opool.tile([128, QB], F16, tag="oT", name=f"oT_{Jb}")
                    o_tiles[Jb] = oT
                else:
                    oT = o_tiles[Jb]
                l_sb = spool.tile([1, QB], F32, tag="lsb")
                nc.vector.tensor_copy(l_sb[:], o_ps[64:65, :])
                rb = spool.tile([1, QB], F32, tag="rb")
                nc.vector.reciprocal_approx_fast(rb[:], l_sb[:])
                bc = spool.tile([64, QB], F32, tag="bc")
                nc.gpsimd.partition_broadcast(bc[:], rb[:])
                nc.vector.tensor_mul(oT[r : r + 64, :], o_ps[0:64, :], bc[:])

            fillers.extend((-1, p) for p in y_pieces(Jb))

        while fillers:
            fillers.popleft()[1]()

    nc.compile()
    return nc


# ---------------- host-side wrapper ----------------

_CACHE = {}


def _get_nc(T):
    if T not in _CACHE:
        _CACHE[T] = build_kernel(T)
    return _CACHE[T]


def _host_prep(x, cos, sin, Wq, Wk, Wv, Wp):
    T = x.shape[1]
    cosT = np.ascontiguousarray(cos.T).astype(np.float32)  # [32, T]
    sinT = np.ascontiguousarray(sin.T).astype(np.float32)
    ccT = np.concatenate([cosT] * 4, axis=0).astype(np.float16)  # [128, T]
    sgn = np.where((np.arange(128) % 64) < 32, 1.0, -1.0)[:, None].astype(np.float32)
    ssT = (np.concatenate([sinT] * 4, axis=0) * sgn).astype(np.float16)
    rr = np.arange(128)[:, None]
    cq = np.arange(512)[None, :]
    masks = np.stack(
        [(cq >= 128 * m + rr) for m in range(4)], axis=1
    ).astype(np.float16)  # [128, 4, 512]
    # qb = perm.T @ qa: qb[d] = qa[swap(d)], swap = +-32 within each 64-row head
    dd = np.arange(128)
    swap = np.where((dd % 64) < 32, dd + 32, dd - 32)
    permM = np.zeros((128, 128), np.float16)
    permM[swap, dd] = 1.0

    in_maps = []
    for core in range(N_CORES):
        b, p = core // 4, core % 4
        hs = slice(128 * p, 128 * (p + 1))
        in_maps.append(
            {
                "xT": np.ascontiguousarray(x[b].T.astype(np.float16)),
                "ccT": ccT,
                "ssT": ssT,
                "wqT": np.ascontiguousarray(Wq[hs].T).astype(np.float16),
                "wkT": np.ascontiguousarray(Wk[hs].T).astype(np.float16),
                "wvT": np.ascontiguousarray(Wv[hs].T).astype(np.float16),
                "wpT": np.ascontiguousarray(Wp[:, hs].T.astype(np.float16)),
                "masks": masks,
                "perm": permM,
            }
        )
    return in_maps


def kernel(x, cos, sin, Wq, Wk, Wv, Wp, _trace=False, _nc=None):
    x = np.asarray(x)
    T = x.shape[1]
    nc = _nc if _nc is not None else _get_nc(T)
    in_maps = _host_prep(
        x, np.asarray(cos), np.asarray(sin),
        np.asarray(Wq), np.asarray(Wk), np.asarray(Wv), np.asarray(Wp),
    )
    res = run_bass_kernel_spmd(nc, in_maps, list(range(N_CORES)), trace=_trace)
    y = np.zeros((B, T, C), np.float32)
    for core in range(N_CORES):
        y[core // 4] += res.results[core]["yT"].T.astype(np.float32)
    kernel.last_results = res
    return y


# revision 10
# speedup vs baseline: 1.1501x; 1.1501x over previous
"""Trainium2 Bass kernel for nn_CausalSelfAttention (B=2, T=4096, D=512, H=8, hd=64).

Sharding: batch x head-pair over 8 cores (core i: batch i//4, heads 2*(i%4), 2*(i%4)+1).
Each core computes QKV projection + RoPE + full-T causal attention for its 2 heads and
a partial output projection (row-parallel c_proj); host sums the 4 partials per batch.

Design (v5). Two engine streams dominate and must both stay saturated:
  - ACT (scalar) runs softmax exp over all T^2/2 scores: ~123us of streaming
    at 1 elem/cycle/lane plus per-instruction overhead; exp groups of 2
    128-k-chunks from fp32 PSUM [128,1024].
  - PE runs S = K^T Q chunks, AV accumulation (ones-augmented V gives the
    softmax denominator for free), QKV projections, RoPE rotate-half as one
    [128x128] permutation matmul (sign folded into host-prepped ss), V^T
    computed directly in transposed form, and the y projection.
The S->exp->S chain is decoupled with THREE s-psum buffers (runway of 3
exp groups) so ACT never starves; AV lags LAG groups behind exp. Projections
for block Jb+1 and the y projection of Jb-1 are emitted as filler pieces
between attention groups of block Jb so the in-order PE queue stays dense
and the HAM clock gate stays released. Normalization: 1/l via
reciprocal_approx_fast on an SBUF copy of the PSUM ones-row, gpsimd
partition_broadcast, one TT-mul writing fp16. y is DMA'd out as fp16; the
host only transposes and accumulates the 4 head-pair partials per batch.
PSUM: s 3x2 + o 1 + aux 1 = 8 banks (s tiles [128,1024]f32 = 2 banks each).
"""

import sys

sys.path.insert(0, "/opt/trn_rl_repo")

from collections import deque
from contextlib import ExitStack

import ml_dtypes
import numpy as np

import concourse.bass as bass
import concourse.tile as tile
from concourse import bacc, mybir
from concourse.bass import ts
from concourse.bass_utils import run_bass_kernel_spmd

F32 = mybir.dt.float32
F16 = mybir.dt.float16

B, C, H, HD = 2, 512, 8, 64
N_CORES = 8


def build_kernel(T=4096, n_cores=N_CORES):
    nc = bacc.Bacc(
        "TRN2",
        target_bir_lowering=False,
        debug=False,
        num_devices=n_cores,
    )
    NJ = T // 512
    NK = T // 128
    QB = 512
    NB = T // QB
    LAG = 2
    GS = 2

    xT_d = nc.dram_tensor("xT", [C, T], F16, kind="ExternalInput").ap()
    cc_d = nc.dram_tensor("ccT", [128, T], F16, kind="ExternalInput").ap()
    ss_d = nc.dram_tensor("ssT", [128, T], F16, kind="ExternalInput").ap()
    w_d = {}
    for name in ("wqT", "wkT", "wvT"):
        w_d[name] = nc.dram_tensor(name, [C, 128], F16, kind="ExternalInput").ap()
    wp_d = nc.dram_tensor("wpT", [128, C], F16, kind="ExternalInput").ap()
    msk_d = nc.dram_tensor("masks", [128, 4, QB], F16, kind="ExternalInput").ap()
    perm_d = nc.dram_tensor("perm", [128, 128], F16, kind="ExternalInput").ap()
    y_d = nc.dram_tensor("yT", [C, T], F16, kind="ExternalOutput").ap()
    warm_d = nc.dram_tensor("warm", [1, 4], F32, kind="ExternalOutput").ap()

    SCALE = float(1.0 / np.sqrt(HD))

    with tile.TileContext(nc) as tc, ExitStack() as ctx:
        consts = ctx.enter_context(tc.tile_pool(name="consts", bufs=1))
        big = ctx.enter_context(tc.tile_pool(name="big", bufs=1))
        xpool = ctx.enter_context(tc.tile_pool(name="xpool", bufs=3))
        qpool = ctx.enter_context(tc.tile_pool(name="qpool", bufs=4))
        rpool = ctx.enter_context(tc.tile_pool(name="rpool", bufs=6))
        epool = ctx.enter_context(tc.tile_pool(name="epool", bufs=5))
        opool = ctx.enter_context(tc.tile_pool(name="opool", bufs=3))
        spool = ctx.enter_context(tc.tile_pool(name="small", bufs=4))
        ypool = ctx.enter_context(tc.tile_pool(name="ypool", bufs=2))

        ps_aux = ctx.enter_context(tc.tile_pool(name="ps_aux", bufs=2, space="PSUM"))
        ps_s = ctx.enter_context(tc.tile_pool(name="ps_s", bufs=2, space="PSUM"))
        ps_o = ctx.enter_context(tc.tile_pool(name="ps_o", bufs=2, space="PSUM"))

        # ---- PE warmup burst first: matmuls on a small memset tile release
        # the HAM clock gate while the first DMAs land. Emitted before any
        # other DVE work so the wz memset is at the head of the DVE queue.
        wz = spool.tile([128, 512], F16, tag="wz")
        nc.vector.memset(wz[:], 0.25)
        wu_ps = ps_aux.tile([128, 512], F32, tag="p")
        for _ in range(6):
            nc.tensor.matmul(wu_ps[:], wz[:, 0:128], wz[:], start=True, stop=True)
        # preload the exp table set while ACT is otherwise idle
        wexp = spool.tile([1, 4], F16, tag="wexp")
        nc.scalar.activation(wexp[:], wu_ps[0:1, 0:4],
                             mybir.ActivationFunctionType.Exp, scale=0.001)
        wsink = spool.tile([1, 4], F32, tag="wsink")
        nc.vector.tensor_copy(wsink[:], wu_ps[0:1, 0:4])
        nc.sync.dma_start(warm_d[:], wsink[:])

        # weights + rope tables on the gpsimd DMA queue (parallel with the
        # x loads on the sync queue)
        w_sb = {}
        for name in ("wqT", "wkT", "wvT"):
            w = consts.tile([128, 4, 128], F16, tag=name, name=f"w_{name}")
            nc.gpsimd.dma_start(w[:], w_d[name].rearrange("(c p) m -> p c m", c=4))
            w_sb[name] = w
        cc = consts.tile([128, T], F16, name="cc")
        ss = consts.tile([128, T], F16, name="ss")
        nc.gpsimd.dma_start(cc[:], cc_d[:])
        nc.gpsimd.dma_start(ss[:], ss_d[:])
        perm = consts.tile([128, 128], F16)
        nc.gpsimd.dma_start(perm[:], perm_d[:])

        krT = big.tile([128, T], F16)
        v_aug = big.tile([128, 2, NK, 65], F16)
        nc.vector.memset(v_aug[:], 1.0)

        masks = consts.tile([128, 4, QB], F16, name="masks")
        nc.gpsimd.dma_start(masks[:], msk_d[:])
        w_p = consts.tile([128, C], F16, name="wp")
        nc.gpsimd.dma_start(w_p[:], wp_d[:])

        qr_tiles = {}
        o_tiles = {}

        def proj_pieces(j):
            """Emit-able pieces of the j-th projection block (label j). Each
            piece is a short PE burst; DVE/DMA consumers run while later
            pieces and surrounding attention groups keep the PE busy."""
            jc = ts(j, 512)
            st = {}

            def p_x():
                xc = xpool.tile([128, 4, 512], F16, tag="xc")
                nc.sync.dma_start(xc[:], xT_d.rearrange("(c p) t -> p c t", c=4)[:, :, jc])
                st["xc"] = xc

            def mk_qk(name, out_tag):
                def piece():
                    ps = ps_aux.tile([128, 512], F32, tag="p", name=f"ps_{name}_{j}")
                    for c in range(4):
                        nc.tensor.matmul(
                            ps[:], w_sb[name][:, c, :], st["xc"][:, c, :],
                            start=(c == 0), stop=(c == 3),
                        )
                    a_sb = qpool.tile([128, 512], F16, tag="a")
                    nc.vector.tensor_copy(a_sb[:], ps[:])
                    st[out_tag] = a_sb
                return piece

            def mk_rope(a_tag, out_name):
                def piece():
                    # qb = perm.T @ qa (the rotate-half partition swap on PE);
                    # m2 reads it straight from PSUM
                    b_ps = ps_aux.tile([128, 512], F32, tag="p", name=f"ps_b_{out_name}_{j}")
                    nc.tensor.matmul(b_ps[:], perm[:], st[a_tag][:], start=True, stop=True)
                    m1 = rpool.tile([128, 512], F16, tag="m1")
                    m2 = rpool.tile([128, 512], F16, tag="m2")
                    nc.vector.tensor_mul(m1[:], st[a_tag][:], cc[:, jc])
                    nc.vector.tensor_mul(m2[:], b_ps[:], ss[:, jc])
                    if out_name == "q":
                        qr = qpool.tile([128, 512], F16, tag="qr", name=f"qr_{j}")
                        nc.vector.tensor_add(qr[:], m1[:], m2[:])
                        qr_tiles[j] = qr
                    else:
                        nc.vector.tensor_add(krT[:, jc], m1[:], m2[:])
                return piece

            def p_vt():
                vt_ps = ps_aux.tile([128, 512], F32, tag="p", name=f"ps_vt_{j}")
                for kc in range(4):
                    for c in range(4):
                        nc.tensor.matmul(
                            vt_ps[:, ts(kc, 128)],
                            st["xc"][:, c, ts(kc, 128)], w_sb["wvT"][:, c, :],
                            start=(c == 0), stop=(c == 3),
                        )
                vw = vt_ps[:].rearrange("p (kc h d) -> p kc h d", kc=4, h=2)
                for hh in range(2):
                    nc.vector.tensor_copy(
                        v_aug[:, hh, 4 * j : 4 * j + 4, 0:64], vw[:, :, hh, :]
                    )

            return [
                p_x,
                mk_qk("wqT", "qa"),
                mk_rope("qa", "q"),
                mk_qk("wkT", "ka"),
                mk_rope("ka", "k"),
                p_vt,
            ]

        def y_pieces(Jb):
            jc = ts(Jb, QB)

            def mk(c):
                def piece():
                    oT = o_tiles[Jb]
                    if c == 3:
                        o_tiles.pop(Jb)
                    y_ps = ps_aux.tile([128, QB], F32, tag="p", name=f"ps_y_{Jb}_{c}")
                    nc.tensor.matmul(
                        y_ps[:], w_p[:, ts(c, 128)], oT[:], start=True, stop=True
                    )
                    y_sb = ypool.tile([128, QB], F16, tag="ysb")
                    nc.vector.tensor_copy(y_sb[:], y_ps[:])
                    nc.sync.dma_start(y_d[ts(c, 128), jc], y_sb[:])
                return piece

            return [mk(0), mk(1), mk(2), mk(3)]

        fillers = deque()  # entries: (label, piece)

        def pump(drain_j=None):
            if drain_j is None:
                if fillers:
                    fillers.popleft()[1]()
            else:
                while any(lbl == drain_j for lbl, _ in fillers):
                    fillers.popleft()[1]()

        for piece in proj_pieces(0):
            piece()
        fillers.extend((1, p) for p in proj_pieces(1))

        for Jb in range(NB):
            jc = ts(Jb, QB)
            if Jb + 1 < NJ:
                fillers.extend((Jb + 1, p) for p in proj_pieces(Jb + 1))
            pump(drain_j=Jb)
            nchunks = 4 * (Jb + 1)
            o_ps = {}
            for h in range(2):
                o_ps[h] = ps_o.tile([65, QB], F32, tag="o", name=f"ps_o_{Jb}_{h}")
            e_tiles = {}
            qr = qr_tiles[Jb]

            def emit_av(g):
                e_sb = e_tiles.pop(g)
                for h in range(2):
                    nc.tensor.matmul(
                        o_ps[h][:],
                        v_aug[:, h, g, :],
                        e_sb[:, h, :],
                        start=(g == 0),
                        stop=(g == nchunks - 1),
                    )

            for g in range(nchunks + LAG):
                if g < nchunks:
                    # one 128-k chunk per head per slot; the two heads' S
                    # matmuls use disjoint PE row halves (tile_position 0/64)
                    # and disjoint PSUM banks -> LDWEIGHTS pull ahead and the
                    # pair streams concurrently
                    s_ps = ps_s.tile([128, 2, QB], F32, tag="s", name=f"ps_s_{Jb}_{g}")
                    for h in range(2):
                        r = 64 * h
                        nc.tensor.matmul(
                            s_ps[:, h, :],
                            krT[r : r + 64, ts(g, 128)],
                            qr[r : r + 64, :],
                            start=True,
                            stop=True,
                        )
                    e_sb = epool.tile([128, 2, QB], F16, tag="e")
                    nc.scalar.activation(
                        e_sb[:], s_ps[:], mybir.ActivationFunctionType.Exp,
                        scale=SCALE,
                    )
                    m = g - 4 * Jb
                    if m >= 0:
                        for h in range(2):
                            nc.vector.tensor_mul(
                                e_sb[:, h, :], e_sb[:, h, :], masks[:, m, :]
                            )
                    e_tiles[g] = e_sb
                if g >= LAG:
                    emit_av(g - LAG)
                    if g % 2 == 0:
                        pump()
                else:
                    pump()

            # normalize: oT[h] = o * (1/l)
            oT = opool.tile([128, QB], F16, tag="oT", name=f"oT_{Jb}")
            o_tiles[Jb] = oT
            for h in range(2):
                r = 64 * h
                l_sb = spool.tile([1, QB], F32, tag="lsb")
                nc.vector.tensor_copy(l_sb[:], o_ps[h][64:65, :])
                rb = spool.tile([1, QB], F32, tag="rb")
                nc.vector.reciprocal_approx_fast(rb[:], l_sb[:])
                bc = spool.tile([64, QB], F32, tag="bc")
                nc.gpsimd.partition_broadcast(bc[:], rb[:])
                nc.vector.tensor_mul(oT[r : r + 64, :], o_ps[h][0:64, :], bc[:])

            fillers.extend((-1, p) for p in y_pieces(Jb))

        while fillers:
            fillers.popleft()[1]()

    nc.compile()
    return nc


# ---------------- host-side wrapper ----------------

_CACHE = {}


def _get_nc(T):
    if T not in _CACHE:
        _CACHE[T] = build_kernel(T)
    return _CACHE[T]


def _host_prep(x, cos, sin, Wq, Wk, Wv, Wp):
    T = x.shape[1]
    cosT = np.ascontiguousarray(cos.T).astype(np.float32)  # [32, T]
    sinT = np.ascontiguousarray(sin.T).astype(np.float32)
    ccT = np.concatenate([cosT] * 4, axis=0).astype(np.float16)  # [128, T]
    sgn = np.where((np.arange(128) % 64) < 32, 1.0, -1.0)[:, None].astype(np.float32)
    ssT = (np.concatenate([sinT] * 4, axis=0) * sgn).astype(np.float16)
    rr = np.arange(128)[:, None]
    cq = np.arange(512)[None, :]
    masks = np.stack(
        [(cq >= 128 * m + rr) for m in range(4)], axis=1
    ).astype(np.float16)  # [128, 4, 512]
    # qb = perm.T @ qa: qb[d] = qa[swap(d)], swap = +-32 within each 64-row head
    dd = np.arange(128)
    swap = np.where((dd % 64) < 32, dd + 32, dd - 32)
    permM = np.zeros((128, 128), np.float16)
    permM[swap, dd] = 1.0

    in_maps = []
    for core in range(N_CORES):
        b, p = core // 4, core % 4
        hs = slice(128 * p, 128 * (p + 1))
        in_maps.append(
            {
                "xT": np.ascontiguousarray(x[b].T.astype(np.float16)),
                "ccT": ccT,
                "ssT": ssT,
                "wqT": np.ascontiguousarray(Wq[hs].T).astype(np.float16),
                "wkT": np.ascontiguousarray(Wk[hs].T).astype(np.float16),
                "wvT": np.ascontiguousarray(Wv[hs].T).astype(np.float16),
                "wpT": np.ascontiguousarray(Wp[:, hs].T.astype(np.float16)),
                "masks": masks,
                "perm": permM,
            }
        )
    return in_maps


def kernel(x, cos, sin, Wq, Wk, Wv, Wp, _trace=False, _nc=None):
    x = np.asarray(x)
    T = x.shape[1]
    nc = _nc if _nc is not None else _get_nc(T)
    in_maps = _host_prep(
        x, np.asarray(cos), np.asarray(sin),
        np.asarray(Wq), np.asarray(Wk), np.asarray(Wv), np.asarray(Wp),
    )
    res = run_bass_kernel_spmd(nc, in_maps, list(range(N_CORES)), trace=_trace)
    y = np.zeros((B, T, C), np.float32)
    for core in range(N_CORES):
        y[core // 4] += res.results[core]["yT"].T.astype(np.float32)
    kernel.last_results = res
    return y
